# revision 7
# baseline (speedup 1.0000x reference)
"""GAT-style masked-softmax attention kernel for Trainium2 (8 NeuronCores).

Problem (per batch b of 32):
    e   = leaky_relu(h @ a1 + (g @ a2)^T, 0.2)        # (N, M)
    att = softmax(where(adj > 0, e, -9e15), axis=-1)  # (N, M)
    out = (att * adj.sum(-1, keepdims=True)) @ g      # (N, D)

Strategy (pure data parallel over B=32 -> 4 batches/core):
  * Mask folded into the scores: softmax(where(a>0, e, -inf)) ==
    softmax(leaky_relu(u_i + v_j + BETA*a_ij - BETA)) for large BETA,
    because Prelu of a hugely negative number stays hugely negative and
    exp() of it underflows to ~0 relative to real scores.
  * adj is cast int32->bf16 during the HBM load (SWDGE cast-on-load),
    landing in natural [i, j] layout. No on-chip transpose of the mask:
    BETA*a^T is produced directly in PSUM by TensorE matmuls of af
    blocks against a BETA*I identity (transpose-by-matmul), accumulated
    with the broadcast of u (K=1 ones matmuls, exact bf16 hi+lo split).
  * ACT then does Prelu(B + (v_j - BETA)) and Exp -> s^T bf16; s^T feeds
    the output matmul (weight-stationary g) and a ones-matmul rowsum.
  * deg_i = sum_j a_ij rides a DVE tensor_scalar accumulate over af
    (natural layout, free-axis reduction); deg and u bounce through DRAM
    once per batch to become row vectors (bf16 hi/lo pairs).
  * Epilogue scales out^T by deg/rowsum row-wise; output stored bf16
    transposed, host un-transposes and casts to f32.

Self-contained: hardcodes shapes B,N,M,D = 32,1024,1024,128 on 8 cores.
"""

import sys

if "/opt/trn_rl_repo" not in sys.path:
    sys.path.insert(0, "/opt/trn_rl_repo")

import numpy as np
import ml_dtypes

import concourse.bacc as bacc
import concourse.mybir as mybir
import concourse.tile as tile
import concourse.bass_utils as bass_utils

F32 = mybir.dt.float32
F32R = mybir.dt.float32r
BF16 = mybir.dt.bfloat16
I32 = mybir.dt.int32
OP = mybir.AluOpType
AF = mybir.ActivationFunctionType

B, N, M, D = 32, 1024, 1024, 128
NCORES = 8
BPC = B // NCORES  # batches per core
NI = N // 128      # i blocks
NJ = M // 128      # j blocks
BETA = 100.0


def build_bass():
    nc = bacc.Bacc("TRN2", target_bir_lowering=False, debug=False)

    h_in = nc.dram_tensor("input1", [BPC, N, D], F32, kind="ExternalInput").ap()
    g_in = nc.dram_tensor("input2", [BPC, M, D], F32, kind="ExternalInput").ap()
    adj_in = nc.dram_tensor("adj", [BPC, N, M], I32, kind="ExternalInput").ap()
    a1_in = nc.dram_tensor("a1", [D, 1], F32, kind="ExternalInput").ap()
    a2_in = nc.dram_tensor("a2", [D, 1], F32, kind="ExternalInput").ap()
    beye_in = nc.dram_tensor("beye", [128, 128], BF16, kind="ExternalInput").ap()
    # out^T bf16: host transposes (0,2,1) + casts f32 after gather
    out_d = nc.dram_tensor("out", [BPC, D, N], BF16, kind="ExternalOutput").ap()

    # bounce scratch: u row (f32) and deg hi/lo rows (bf16)
    urow_scr = nc.dram_tensor("urow_scr", [BPC, N], F32).ap()
    deg_scr = nc.dram_tensor("deg_scr", [BPC, 2, N], BF16).ap()

    with tile.TileContext(nc) as tc:
        with (
            tc.tile_pool(name="singles", bufs=1) as singles,
            tc.tile_pool(name="hg", bufs=2) as hg_pool,
            tc.tile_pool(name="gbf", bufs=2) as gbf_pool,
            tc.tile_pool(name="afp", bufs=2) as af_pool,
            tc.tile_pool(name="plp", bufs=2) as pl_pool,
            tc.tile_pool(name="st", bufs=2) as st_pool,
            tc.tile_pool(name="small", bufs=4) as small,
            tc.tile_pool(name="folds", bufs=2) as folds,
            tc.tile_pool(name="rows", bufs=2) as rows_pool,
            tc.tile_pool(name="rows1", bufs=1) as rows1_pool,
            tc.tile_pool(name="psB", bufs=2, space="PSUM") as psB,
            tc.tile_pool(name="psO", bufs=1, space="PSUM") as psO,
            tc.tile_pool(name="psR", bufs=1, space="PSUM") as psR,
        ):
            # ---- static prep ------------------------------------------------
            ones_row = singles.tile([1, 128], BF16)
            nc.vector.memset(ones_row[:], 1.0)
            ones_sq_bf = singles.tile([128, 128], BF16)
            nc.vector.memset(ones_sq_bf[:], 1.0)
            beye = singles.tile([128, 128], BF16)
            nc.sync.dma_start(beye[:], beye_in)

            a1row = singles.tile([1, D], F32)
            nc.gpsimd.dma_start(a1row[:], a1_in.transpose((1, 0)))
            a2row = singles.tile([1, D], F32)
            nc.gpsimd.dma_start(a2row[:], a2_in.transpose((1, 0)))
            ones_f = singles.tile([1, 128], F32)
            nc.vector.memset(ones_f[:], 1.0)
            ones_r = singles.tile([1, 128], F32R)
            nc.vector.tensor_copy(ones_r[:], ones_f[:])

            a1bc = singles.tile([128, D], F32)
            a2bc = singles.tile([128, D], F32)
            bc_ps = psB.tile([128, N], F32, tag="B")
            nc.tensor.matmul(
                bc_ps[:, 0:D], ones_f[:], a1row[:], start=True, stop=False
            )
            nc.tensor.matmul(
                bc_ps[:, D : 2 * D], ones_f[:], a2row[:], start=False, stop=True
            )
            nc.vector.tensor_copy(a1bc[:], bc_ps[:, 0:D])
            nc.vector.tensor_copy(a2bc[:], bc_ps[:, D : 2 * D])


            def prefetch(b):
                h_t = hg_pool.tile([128, NI, D], F32, tag="h")
                nc.sync.dma_start(
                    h_t[:], h_in[b].rearrange("(ib p) d -> p ib d", p=128)
                )
                g_t = hg_pool.tile([128, NJ, D], F32, tag="g")
                nc.sync.dma_start(
                    g_t[:], g_in[b].rearrange("(jb p) d -> p jb d", p=128)
                )
                # adj int32 -> bf16 cast during the load (SWDGE), natural layout
                af = af_pool.tile([128, NI, M], BF16)
                nc.gpsimd.dma_start(
                    af[:], adj_in[b].rearrange("(ib p) m -> p ib m", p=128)
                )
                return h_t, g_t, af

            def stage_uv(b, h_t, g_t, af):
                # bf16 g for the output matmul
                g_bf = gbf_pool.tile([128, NJ, D], BF16)
                nc.vector.tensor_copy(g_bf[:], g_t[:])
                # u/v projections (u_i = h_i . a1, v_j = g_j . a2)
                ucols = small.tile([128, NI], F32, tag="ucols")
                vcols = small.tile([128, NJ], F32, tag="vcols")
                uscr = small.tile([128, D], F32, tag="uscr")
                for ib in range(NI):
                    nc.vector.scalar_tensor_tensor(
                        uscr[:], h_t[:, ib, :], 0.0, a1bc[:],
                        OP.bypass, OP.mult, accum_out=ucols[:, ib : ib + 1],
                    )
                for jb in range(NJ):
                    nc.vector.scalar_tensor_tensor(
                        uscr[:], g_t[:, jb, :], 0.0, a2bc[:],
                        OP.bypass, OP.mult, accum_out=vcols[:, jb : jb + 1],
                    )
                # bias for the Prelu pass: v_j - BETA (per-partition)
                biasv = small.tile([128, NJ], F32, tag="biasv")
                nc.vector.tensor_scalar(
                    biasv[:], vcols[:], BETA, None, OP.subtract
                )
                # deg_i = sum_j a_ij: per-ib pairwise folds on gpsimd
                # (values stay small ints, exact in bf16), one DVE reduce
                sC = folds.tile([128, NI, 128], BF16, tag="sC")
                for ib in range(NI):
                    sA = folds.tile([128, 512], BF16, tag="sA")
                    nc.gpsimd.tensor_tensor(
                        sA[:], af[:, ib, 0:512], af[:, ib, 512:1024], OP.add
                    )
                    sB = folds.tile([128, 256], BF16, tag="sB")
                    nc.gpsimd.tensor_tensor(
                        sB[:], sA[:, 0:256], sA[:, 256:512], OP.add
                    )
                    nc.gpsimd.tensor_tensor(
                        sC[:, ib], sB[:, 0:128], sB[:, 128:256], OP.add
                    )
                degc = small.tile([128, NI], F32, tag="degc")
                nc.vector.tensor_reduce(
                    degc[:], sC[:], mybir.AxisListType.X, OP.add
                )
                # exact bf16 hi/lo split of deg columns
                c2 = small.tile([128, 2, NI], BF16, tag="c2")
                nc.vector.tensor_copy(c2[:, 0], degc[:])
                nc.vector.tensor_tensor(c2[:, 1], degc[:], c2[:, 0], OP.subtract)
                # bounce through DRAM -> row vectors (scalar HWDGE queue:
                # separate ring, never queues behind the big loads)
                nc.scalar.dma_start(
                    urow_scr[b].rearrange("(ib p) -> p ib", p=128), ucols[:]
                )
                nc.scalar.dma_start(
                    deg_scr[b].rearrange("r (ib p) -> p r ib", p=128), c2[:]
                )
                drows = rows_pool.tile([1, 2, N], BF16, tag="drows")
                nc.scalar.dma_start(drows[:], deg_scr[b].unsqueeze(0))
                # u row straight to f32r (gpsimd cast path rounds on load)
                urow_r = rows_pool.tile([1, N], F32R, tag="urow_r")
                nc.gpsimd.dma_start(urow_r[:], urow_scr[b].unsqueeze(0))
                return g_bf, biasv, urow_r, drows

            pf = prefetch(0)
            uv = stage_uv(0, *pf)
            for b in range(BPC):
                h_t, g_t, af = pf
                g_bf, biasv, urow_r, drows = uv

                # next batch: loads + uv/deg/bounce emitted first so those
                # pipelines run a full batch ahead
                if b + 1 < BPC:
                    pf = prefetch(b + 1)
                    uv = stage_uv(b + 1, *pf)

                outT_ps = psO.tile([128, N], F32, tag="o")
                rs_ps = psR.tile([128, N], F32, tag="r")
                sT = st_pool.tile([128, NJ, N], BF16)
                pl = None
                for jb in range(NJ):
                    # scores^T for block jb: B[j', i] = BETA*a[i, j] + u_i
                    B_ps = psB.tile([128, N], F32, tag="B")
                    for ib in range(NI):
                        nc.tensor.matmul(
                            B_ps[:, ib * 128 : (ib + 1) * 128],
                            af[:, ib, jb * 128 : (jb + 1) * 128],
                            beye[:],
                            start=(ib in (0, 4)), stop=False,
                        )
                    for half in range(2):
                        fs = slice(half * 512, (half + 1) * 512)
                        nc.tensor.matmul(
                            B_ps[:, fs], ones_r[:], urow_r[:, fs],
                            start=False, stop=(half == 1),
                        )
                    if jb % 4 == 0:
                        pl = pl_pool.tile([128, 4, N], F32)
                    nc.scalar.activation(
                        pl[:, jb % 4, :], B_ps[:], AF.Prelu,
                        bias=biasv[:, jb : jb + 1], alpha=0.2,
                    )
                    if jb % 4 == 3:
                        nc.scalar.activation(
                            sT[:, jb - 3 : jb + 1, :], pl[:], AF.Exp
                        )
                        for j2 in range(jb - 3, jb + 1):
                            for half in range(2):
                                fs = slice(half * 512, (half + 1) * 512)
                                nc.tensor.matmul(
                                    outT_ps[:, fs], g_bf[:, j2, :],
                                    sT[:, j2, fs],
                                    start=(j2 == 0), stop=(j2 == NJ - 1),
                                )
                                nc.tensor.matmul(
                                    rs_ps[:, fs], ones_sq_bf[:],
                                    sT[:, j2, fs],
                                    start=(j2 == 0), stop=(j2 == NJ - 1),
                                )

                # ---- epilogue: out^T * (deg / rowsum), all row-form ---------
                rrow = rows1_pool.tile([128, N], F32, tag="rrow")
                nc.vector.reciprocal_approx_fast(rrow[:], rs_ps[:])
                deg_ps = psB.tile([128, N], F32, tag="B")
                for half in range(2):
                    fs = slice(half * 512, (half + 1) * 512)
                    nc.tensor.matmul(
                        deg_ps[:, fs], ones_row[:], drows[:, 0, fs],
                        start=True, stop=False,
                    )
                    nc.tensor.matmul(
                        deg_ps[:, fs], ones_row[:], drows[:, 1, fs],
                        start=False, stop=(half == 1),
                    )
                fac = rows1_pool.tile([128, N], F32, tag="fac")
                nc.vector.tensor_tensor(fac[:], deg_ps[:], rrow[:], OP.mult)
                outsbT = rows1_pool.tile([128, N], BF16, tag="outsbT")
                nc.vector.tensor_tensor(outsbT[:], outT_ps[:], fac[:], OP.mult)
                nc.sync.dma_start(out_d[b], outsbT[:])

    nc.compile()
    return nc


_CACHE = {}


def _get_nc():
    if "nc" not in _CACHE:
        _CACHE["nc"] = build_bass()
    return _CACHE["nc"]


def _make_in_maps(input1, input2, adj, a1, a2):
    input1 = np.ascontiguousarray(np.asarray(input1, dtype=np.float32))
    input2 = np.ascontiguousarray(np.asarray(input2, dtype=np.float32))
    adj = np.ascontiguousarray(np.asarray(adj, dtype=np.int32))
    a1 = np.ascontiguousarray(np.asarray(a1, dtype=np.float32))
    a2 = np.ascontiguousarray(np.asarray(a2, dtype=np.float32))
    beye = np.ascontiguousarray((np.eye(128) * BETA).astype(ml_dtypes.bfloat16))
    in_maps = []
    for c in range(NCORES):
        sl = slice(c * BPC, (c + 1) * BPC)
        in_maps.append(
            {
                "input1": input1[sl],
                "input2": input2[sl],
                "adj": adj[sl],
                "a1": a1,
                "a2": a2,
                "beye": beye,
            }
        )
    return in_maps


def _gather(res):
    # device emits out^T (BPC, D, N) bf16; un-transpose + cast (layout only)
    return np.concatenate(
        [
            np.asarray(r["out"]).astype(np.float32).transpose(0, 2, 1)
            for r in res.results
        ],
        axis=0,
    )


def kernel(input1, input2, adj, a1, a2):
    nc = _get_nc()
    res = bass_utils.run_bass_kernel_spmd(
        nc, _make_in_maps(input1, input2, adj, a1, a2),
        core_ids=list(range(NCORES)),
    )
    return _gather(res)


def run_traced(input1, input2, adj, a1, a2, trace_cores=None):
    nc = _get_nc()
    res = bass_utils.run_bass_kernel_spmd(
        nc, _make_in_maps(input1, input2, adj, a1, a2),
        core_ids=list(range(NCORES)),
        trace=True,
        trace_cores=trace_cores or [0],
    )
    return _gather(res), res


# revision 8
# speedup vs baseline: 1.0381x; 1.0381x over previous
"""GAT-style masked-softmax attention kernel for Trainium2 (8 NeuronCores).

Problem (per batch b of 32):
    e   = leaky_relu(h @ a1 + (g @ a2)^T, 0.2)        # (N, M)
    att = softmax(where(adj > 0, e, -9e15), axis=-1)  # (N, M)
    out = (att * adj.sum(-1, keepdims=True)) @ g      # (N, D)

Strategy (pure data parallel over B=32 -> 4 batches/core):
  * Mask folded into the scores: softmax(where(a>0, e, -inf)) ==
    softmax(leaky_relu(u_i + v_j + BETA*a_ij - BETA)) for large BETA,
    because Prelu of a hugely negative number stays hugely negative and
    exp() of it underflows to ~0 relative to real scores.
  * adj is cast int32->bf16 during the HBM load (SWDGE cast-on-load),
    landing in natural [i, j] layout. No on-chip transpose of the mask:
    BETA*a^T is produced directly in PSUM by TensorE matmuls of af
    blocks against a BETA*I identity (transpose-by-matmul), accumulated
    with the broadcast of u (K=1 ones matmuls, exact bf16 hi+lo split).
  * ACT then does Prelu(B + (v_j - BETA)) and Exp -> s^T bf16; s^T feeds
    the output matmul (weight-stationary g) and a ones-matmul rowsum.
  * deg_i = sum_j a_ij rides a DVE tensor_scalar accumulate over af
    (natural layout, free-axis reduction); deg and u bounce through DRAM
    once per batch to become row vectors (bf16 hi/lo pairs).
  * Epilogue scales out^T by deg/rowsum row-wise; output stored bf16
    transposed, host un-transposes and casts to f32.

Self-contained: hardcodes shapes B,N,M,D = 32,1024,1024,128 on 8 cores.
"""

import sys

if "/opt/trn_rl_repo" not in sys.path:
    sys.path.insert(0, "/opt/trn_rl_repo")

import numpy as np
import ml_dtypes

import concourse.bacc as bacc
import concourse.mybir as mybir
import concourse.tile as tile
import concourse.bass_utils as bass_utils

F32 = mybir.dt.float32
F32R = mybir.dt.float32r
BF16 = mybir.dt.bfloat16
I32 = mybir.dt.int32
OP = mybir.AluOpType
AF = mybir.ActivationFunctionType

B, N, M, D = 32, 1024, 1024, 128
NCORES = 8
BPC = B // NCORES  # batches per core
NI = N // 128      # i blocks
NJ = M // 128      # j blocks
BETA = 100.0


def build_bass():
    nc = bacc.Bacc("TRN2", target_bir_lowering=False, debug=False)

    h_in = nc.dram_tensor("input1", [BPC, N, D], F32, kind="ExternalInput").ap()
    g_in = nc.dram_tensor("input2", [BPC, M, D], F32, kind="ExternalInput").ap()
    adj_in = nc.dram_tensor("adj", [BPC, N, M], I32, kind="ExternalInput").ap()
    a1_in = nc.dram_tensor("a1", [D, 1], F32, kind="ExternalInput").ap()
    a2_in = nc.dram_tensor("a2", [D, 1], F32, kind="ExternalInput").ap()
    beye_in = nc.dram_tensor("beye", [128, 128], BF16, kind="ExternalInput").ap()
    # out^T bf16: host transposes (0,2,1) + casts f32 after gather
    out_d = nc.dram_tensor("out", [BPC, D, N], BF16, kind="ExternalOutput").ap()

    # bounce scratch: u row (f32) and deg hi/lo rows (bf16)
    urow_scr = nc.dram_tensor("urow_scr", [BPC, N], F32R).ap()
    deg_scr = nc.dram_tensor("deg_scr", [BPC, 2, N], BF16).ap()

    with tile.TileContext(nc) as tc:
        with (
            tc.tile_pool(name="singles", bufs=1) as singles,
            tc.tile_pool(name="hg", bufs=2) as hg_pool,
            tc.tile_pool(name="gbf", bufs=2) as gbf_pool,
            tc.tile_pool(name="afp", bufs=2) as af_pool,
            tc.tile_pool(name="plp", bufs=2) as pl_pool,
            tc.tile_pool(name="st", bufs=2) as st_pool,
            tc.tile_pool(name="small", bufs=4) as small,
            tc.tile_pool(name="folds", bufs=2) as folds,
            tc.tile_pool(name="rows", bufs=2) as rows_pool,
            tc.tile_pool(name="rows1", bufs=1) as rows1_pool,
            tc.tile_pool(name="psB", bufs=2, space="PSUM") as psB,
            tc.tile_pool(name="psO", bufs=1, space="PSUM") as psO,
            tc.tile_pool(name="psR", bufs=1, space="PSUM") as psR,
        ):
            # ---- static prep ------------------------------------------------
            ones_row = singles.tile([1, 128], BF16)
            nc.vector.memset(ones_row[:], 1.0)
            ones_sq_bf = singles.tile([128, 128], BF16)
            nc.vector.memset(ones_sq_bf[:], 1.0)
            beye = singles.tile([128, 128], BF16)
            nc.sync.dma_start(beye[:], beye_in)

            a1row = singles.tile([1, D], F32)
            nc.gpsimd.dma_start(a1row[:], a1_in.transpose((1, 0)))
            a2row = singles.tile([1, D], F32)
            nc.gpsimd.dma_start(a2row[:], a2_in.transpose((1, 0)))
            ones_f = singles.tile([1, 128], F32)
            nc.vector.memset(ones_f[:], 1.0)
            ones_r = singles.tile([1, 128], F32R)
            nc.vector.tensor_copy(ones_r[:], ones_f[:])

            a1bc = singles.tile([128, D], F32)
            a2bc = singles.tile([128, D], F32)
            bc_ps = psB.tile([128, N], F32, tag="B")
            nc.tensor.matmul(
                bc_ps[:, 0:D], ones_f[:], a1row[:], start=True, stop=False
            )
            nc.tensor.matmul(
                bc_ps[:, D : 2 * D], ones_f[:], a2row[:], start=False, stop=True
            )
            nc.vector.tensor_copy(a1bc[:], bc_ps[:, 0:D])
            nc.vector.tensor_copy(a2bc[:], bc_ps[:, D : 2 * D])


            def prefetch(b):
                h_t = hg_pool.tile([128, NI, D], F32, tag="h")
                nc.sync.dma_start(
                    h_t[:], h_in[b].rearrange("(ib p) d -> p ib d", p=128)
                )
                g_t = hg_pool.tile([128, NJ, D], F32, tag="g")
                nc.sync.dma_start(
                    g_t[:], g_in[b].rearrange("(jb p) d -> p jb d", p=128)
                )
                # adj int32 -> bf16 cast during the load (SWDGE), natural layout
                af = af_pool.tile([128, NI, M], BF16)
                nc.gpsimd.dma_start(
                    af[:], adj_in[b].rearrange("(ib p) m -> p ib m", p=128)
                )
                return h_t, g_t, af

            def stage_uv(b, h_t, g_t, af):
                # bf16 g for the output matmul
                g_bf = gbf_pool.tile([128, NJ, D], BF16)
                nc.vector.tensor_copy(g_bf[:], g_t[:])
                # u/v projections (u_i = h_i . a1, v_j = g_j . a2)
                ucols = small.tile([128, NI], F32, tag="ucols")
                vcols = small.tile([128, NJ], F32, tag="vcols")
                uscr = small.tile([128, D], F32, tag="uscr")
                for ib in range(NI):
                    nc.vector.scalar_tensor_tensor(
                        uscr[:], h_t[:, ib, :], 0.0, a1bc[:],
                        OP.bypass, OP.mult, accum_out=ucols[:, ib : ib + 1],
                    )
                for jb in range(NJ):
                    nc.vector.scalar_tensor_tensor(
                        uscr[:], g_t[:, jb, :], 0.0, a2bc[:],
                        OP.bypass, OP.mult, accum_out=vcols[:, jb : jb + 1],
                    )
                # bias for the Prelu pass: v_j - BETA (per-partition)
                biasv = small.tile([128, NJ], F32, tag="biasv")
                nc.vector.tensor_scalar(
                    biasv[:], vcols[:], BETA, None, OP.subtract
                )
                # deg_i = sum_j a_ij: 8 accumulate-DMAs fold j into 128
                # partials (SWDGE CCE add; sums <= 8, exact in bf16), then
                # one small DVE reduce
                dacc = folds.tile([128, NI, 128], BF16, tag="dacc")
                nc.gpsimd.dma_start(dacc[:], af[:, :, 0:128])
                for t in range(1, NI):
                    nc.gpsimd.dma_start(
                        dacc[:], af[:, :, t * 128 : (t + 1) * 128],
                        accum_op=OP.add,
                    )
                degc = small.tile([128, NI], F32, tag="degc")
                nc.vector.tensor_reduce(
                    degc[:], dacc[:], mybir.AxisListType.X, OP.add
                )
                # exact bf16 hi/lo split of deg columns
                c2 = small.tile([128, 2, NI], BF16, tag="c2")
                nc.vector.tensor_copy(c2[:, 0], degc[:])
                nc.vector.tensor_tensor(c2[:, 1], degc[:], c2[:, 0], OP.subtract)
                # bounce through DRAM -> row vectors (scalar HWDGE queue:
                # separate ring, never queues behind the big loads)
                ucols_r = small.tile([128, NI], F32R, tag="ucols_r")
                nc.vector.tensor_copy(ucols_r[:], ucols[:])
                nc.scalar.dma_start(
                    urow_scr[b].rearrange("(ib p) -> p ib", p=128), ucols_r[:]
                )
                nc.scalar.dma_start(
                    deg_scr[b].rearrange("r (ib p) -> p r ib", p=128), c2[:]
                )
                drows = rows_pool.tile([1, 2, N], BF16, tag="drows")
                nc.scalar.dma_start(drows[:], deg_scr[b].unsqueeze(0))
                # u row: f32r bounce entirely on the scalar HWDGE queue
                urow_r = rows_pool.tile([1, N], F32R, tag="urow_r")
                nc.scalar.dma_start(urow_r[:], urow_scr[b].unsqueeze(0))
                return g_bf, biasv, urow_r, drows

            pf = prefetch(0)
            uv = stage_uv(0, *pf)
            for b in range(BPC):
                h_t, g_t, af = pf
                g_bf, biasv, urow_r, drows = uv

                # next batch: loads + uv/deg/bounce emitted first so those
                # pipelines run a full batch ahead
                if b + 1 < BPC:
                    pf = prefetch(b + 1)
                    uv = stage_uv(b + 1, *pf)

                outT_ps = psO.tile([128, N], F32, tag="o")
                rs_ps = psR.tile([128, N], F32, tag="r")
                sT = st_pool.tile([128, NJ, N], BF16)
                pl = None
                for jb in range(NJ):
                    # scores^T for block jb: B[j', i] = BETA*a[i, j] + u_i
                    B_ps = psB.tile([128, N], F32, tag="B")
                    for ib in range(NI):
                        nc.tensor.matmul(
                            B_ps[:, ib * 128 : (ib + 1) * 128],
                            af[:, ib, jb * 128 : (jb + 1) * 128],
                            beye[:],
                            start=(ib in (0, 4)), stop=False,
                        )
                    for half in range(2):
                        fs = slice(half * 512, (half + 1) * 512)
                        nc.tensor.matmul(
                            B_ps[:, fs], ones_r[:], urow_r[:, fs],
                            start=False, stop=(half == 1),
                        )
                    if jb % 4 == 0:
                        pl = pl_pool.tile([128, 4, N], F32)
                    nc.scalar.activation(
                        pl[:, jb % 4, :], B_ps[:], AF.Prelu,
                        bias=biasv[:, jb : jb + 1], alpha=0.2,
                    )
                    if jb % 4 == 3:
                        nc.scalar.activation(
                            sT[:, jb - 3 : jb + 1, :], pl[:], AF.Exp
                        )
                        for j2 in range(jb - 3, jb + 1):
                            for half in range(2):
                                fs = slice(half * 512, (half + 1) * 512)
                                nc.tensor.matmul(
                                    outT_ps[:, fs], g_bf[:, j2, :],
                                    sT[:, j2, fs],
                                    start=(j2 == 0), stop=(j2 == NJ - 1),
                                )
                                nc.tensor.matmul(
                                    rs_ps[:, fs], ones_sq_bf[:],
                                    sT[:, j2, fs],
                                    start=(j2 == 0), stop=(j2 == NJ - 1),
                                )

                # ---- epilogue: out^T * (deg / rowsum), all row-form ---------
                rrow = rows1_pool.tile([128, N], F32, tag="rrow")
                nc.vector.reciprocal_approx_fast(rrow[:], rs_ps[:])
                deg_ps = psB.tile([128, N], F32, tag="B")
                for half in range(2):
                    fs = slice(half * 512, (half + 1) * 512)
                    nc.tensor.matmul(
                        deg_ps[:, fs], ones_row[:], drows[:, 0, fs],
                        start=True, stop=False,
                    )
                    nc.tensor.matmul(
                        deg_ps[:, fs], ones_row[:], drows[:, 1, fs],
                        start=False, stop=(half == 1),
                    )
                fac = rows1_pool.tile([128, N], F32, tag="fac")
                nc.vector.tensor_tensor(fac[:], deg_ps[:], rrow[:], OP.mult)
                outsbT = rows1_pool.tile([128, N], BF16, tag="outsbT")
                nc.vector.tensor_tensor(outsbT[:], outT_ps[:], fac[:], OP.mult)
                nc.sync.dma_start(out_d[b], outsbT[:])

    nc.compile()
    return nc


_CACHE = {}


def _get_nc():
    if "nc" not in _CACHE:
        _CACHE["nc"] = build_bass()
    return _CACHE["nc"]


def _make_in_maps(input1, input2, adj, a1, a2):
    input1 = np.ascontiguousarray(np.asarray(input1, dtype=np.float32))
    input2 = np.ascontiguousarray(np.asarray(input2, dtype=np.float32))
    adj = np.ascontiguousarray(np.asarray(adj, dtype=np.int32))
    a1 = np.ascontiguousarray(np.asarray(a1, dtype=np.float32))
    a2 = np.ascontiguousarray(np.asarray(a2, dtype=np.float32))
    beye = np.ascontiguousarray((np.eye(128) * BETA).astype(ml_dtypes.bfloat16))
    in_maps = []
    for c in range(NCORES):
        sl = slice(c * BPC, (c + 1) * BPC)
        in_maps.append(
            {
                "input1": input1[sl],
                "input2": input2[sl],
                "adj": adj[sl],
                "a1": a1,
                "a2": a2,
                "beye": beye,
            }
        )
    return in_maps


def _gather(res):
    # device emits out^T (BPC, D, N) bf16; un-transpose + cast (layout only)
    return np.concatenate(
        [
            np.asarray(r["out"]).astype(np.float32).transpose(0, 2, 1)
            for r in res.results
        ],
        axis=0,
    )


def kernel(input1, input2, adj, a1, a2):
    nc = _get_nc()
    res = bass_utils.run_bass_kernel_spmd(
        nc, _make_in_maps(input1, input2, adj, a1, a2),
        core_ids=list(range(NCORES)),
    )
    return _gather(res)


def run_traced(input1, input2, adj, a1, a2, trace_cores=None):
    nc = _get_nc()
    res = bass_utils.run_bass_kernel_spmd(
        nc, _make_in_maps(input1, input2, adj, a1, a2),
        core_ids=list(range(NCORES)),
        trace=True,
        trace_cores=trace_cores or [0],
    )
    return _gather(res), res


# revision 9
# speedup vs baseline: 1.1274x; 1.0860x over previous
"""GAT-style masked-softmax attention kernel for Trainium2 (8 NeuronCores).

Problem (per batch b of 32):
    e   = leaky_relu(h @ a1 + (g @ a2)^T, 0.2)        # (N, M)
    att = softmax(where(adj > 0, e, -9e15), axis=-1)  # (N, M)
    out = (att * adj.sum(-1, keepdims=True)) @ g      # (N, D)

Strategy (pure data parallel over B=32 -> 4 batches/core):
  * Mask folded into the scores: softmax(where(a>0, e, -inf)) ==
    softmax(leaky_relu(u_i + v_j + BETA*a_ij - BETA)) for large BETA,
    because Prelu of a hugely negative number stays hugely negative and
    exp() of it underflows to ~0 relative to real scores.
  * adj is cast int32->bf16 during the HBM load (SWDGE cast-on-load),
    landing in natural [i, j] layout. No on-chip transpose of the mask:
    BETA*a^T is produced directly in PSUM by TensorE matmuls of af
    blocks against a BETA*I identity (transpose-by-matmul), accumulated
    with the broadcast of u (K=1 ones matmuls, exact bf16 hi+lo split).
  * ACT then does Prelu(B + (v_j - BETA)) and Exp -> s^T bf16; s^T feeds
    the output matmul (weight-stationary g) and a ones-matmul rowsum.
  * deg_i = sum_j a_ij rides a DVE tensor_scalar accumulate over af
    (natural layout, free-axis reduction); deg and u bounce through DRAM
    once per batch to become row vectors (bf16 hi/lo pairs).
  * Epilogue scales out^T by deg/rowsum row-wise; output stored bf16
    transposed, host un-transposes and casts to f32.

Self-contained: hardcodes shapes B,N,M,D = 32,1024,1024,128 on 8 cores.
"""

import sys

if "/opt/trn_rl_repo" not in sys.path:
    sys.path.insert(0, "/opt/trn_rl_repo")

import numpy as np
import ml_dtypes

import concourse.bacc as bacc
import concourse.mybir as mybir
import concourse.tile as tile
import concourse.bass_utils as bass_utils

F32 = mybir.dt.float32
F32R = mybir.dt.float32r
BF16 = mybir.dt.bfloat16
I32 = mybir.dt.int32
OP = mybir.AluOpType
AF = mybir.ActivationFunctionType

B, N, M, D = 32, 1024, 1024, 128
NCORES = 8
BPC = B // NCORES  # batches per core
NI = N // 128      # i blocks
NJ = M // 128      # j blocks
BETA = 100.0


def build_bass():
    nc = bacc.Bacc("TRN2", target_bir_lowering=False, debug=False)

    h_in = nc.dram_tensor("input1", [BPC, N, D], F32, kind="ExternalInput").ap()
    g_in = nc.dram_tensor("input2", [BPC, M, D], F32, kind="ExternalInput").ap()
    adj_in = nc.dram_tensor("adj", [BPC, N, M], I32, kind="ExternalInput").ap()
    a1_in = nc.dram_tensor("a1", [D, 1], F32, kind="ExternalInput").ap()
    a2_in = nc.dram_tensor("a2", [D, 1], F32, kind="ExternalInput").ap()
    beye_in = nc.dram_tensor("beye", [128, 128], BF16, kind="ExternalInput").ap()
    # out^T bf16: host transposes (0,2,1) + casts f32 after gather
    out_d = nc.dram_tensor("out", [BPC, D, N], BF16, kind="ExternalOutput").ap()

    # bounce scratch: u row (f32) and deg hi/lo rows (bf16)
    urow_scr = nc.dram_tensor("urow_scr", [BPC, N], F32R).ap()
    deg_scr = nc.dram_tensor("deg_scr", [BPC, 2, N], BF16).ap()

    with tile.TileContext(nc) as tc:
        with (
            tc.tile_pool(name="singles", bufs=1) as singles,
            tc.tile_pool(name="hg", bufs=2) as hg_pool,
            tc.tile_pool(name="gbf", bufs=2) as gbf_pool,
            tc.tile_pool(name="afp", bufs=2) as af_pool,
            tc.tile_pool(name="plp", bufs=2) as pl_pool,
            tc.tile_pool(name="st", bufs=2) as st_pool,
            tc.tile_pool(name="small", bufs=4) as small,
            tc.tile_pool(name="folds", bufs=2) as folds,
            tc.tile_pool(name="rows", bufs=2) as rows_pool,
            tc.tile_pool(name="rows1", bufs=1) as rows1_pool,
            tc.tile_pool(name="psB", bufs=2, space="PSUM") as psB,
            tc.tile_pool(name="psO", bufs=1, space="PSUM") as psO,
            tc.tile_pool(name="psR", bufs=1, space="PSUM") as psR,
        ):
            # ---- static prep ------------------------------------------------
            ones_row = singles.tile([1, 128], BF16)
            nc.vector.memset(ones_row[:], 1.0)
            ones_sq_bf = singles.tile([128, 128], BF16)
            nc.vector.memset(ones_sq_bf[:], 1.0)
            beye = singles.tile([128, 128], BF16)
            nc.sync.dma_start(beye[:], beye_in)

            a1row = singles.tile([1, D], F32)
            nc.gpsimd.dma_start(a1row[:], a1_in.transpose((1, 0)))
            a2row = singles.tile([1, D], F32)
            nc.gpsimd.dma_start(a2row[:], a2_in.transpose((1, 0)))
            ones_f = singles.tile([1, 128], F32)
            nc.vector.memset(ones_f[:], 1.0)
            ones_r = singles.tile([1, 128], F32R)
            nc.vector.tensor_copy(ones_r[:], ones_f[:])

            a1bc = singles.tile([128, D], F32)
            a2bc = singles.tile([128, D], F32)
            bc_ps = psB.tile([128, N], F32, tag="B")
            nc.tensor.matmul(
                bc_ps[:, 0:D], ones_f[:], a1row[:], start=True, stop=False
            )
            nc.tensor.matmul(
                bc_ps[:, D : 2 * D], ones_f[:], a2row[:], start=False, stop=True
            )
            nc.vector.tensor_copy(a1bc[:], bc_ps[:, 0:D])
            nc.vector.tensor_copy(a2bc[:], bc_ps[:, D : 2 * D])


            def prefetch(b):
                h_t = hg_pool.tile([128, NI, D], F32, tag="h")
                nc.sync.dma_start(
                    h_t[:], h_in[b].rearrange("(ib p) d -> p ib d", p=128)
                )
                g_t = hg_pool.tile([128, NJ, D], F32, tag="g")
                nc.sync.dma_start(
                    g_t[:], g_in[b].rearrange("(jb p) d -> p jb d", p=128)
                )
                # adj int32 -> bf16 cast during the load (SWDGE), natural layout
                af = af_pool.tile([128, NI, M], BF16)
                nc.gpsimd.dma_start(
                    af[:], adj_in[b].rearrange("(ib p) m -> p ib m", p=128)
                )
                return h_t, g_t, af

            def stage_deg_head(b, af):
                # deg_i = sum_j a_ij for the CURRENT batch: af is already
                # resident, so these SWDGE accumulate-DMAs issue without
                # blocking the gpsimd stream (sums <= 8, exact in bf16)
                dacc = folds.tile([128, NI, 128], BF16, tag="dacc")
                nc.gpsimd.dma_start(dacc[:], af[:, :, 0:128])
                for t in range(1, NI):
                    nc.gpsimd.dma_start(
                        dacc[:], af[:, :, t * 128 : (t + 1) * 128],
                        accum_op=OP.add,
                    )
                degc = small.tile([128, NI], F32, tag="degc")
                nc.vector.tensor_reduce(
                    degc[:], dacc[:], mybir.AxisListType.X, OP.add
                )
                # exact bf16 hi/lo split; store on the sync queue
                c2 = small.tile([128, 2, NI], BF16, tag="c2")
                nc.vector.tensor_copy(c2[:, 0], degc[:])
                nc.vector.tensor_tensor(c2[:, 1], degc[:], c2[:, 0], OP.subtract)
                nc.sync.dma_start(
                    deg_scr[b].rearrange("r (ib p) -> p r ib", p=128), c2[:]
                )

            def stage_deg_tail(b):
                # row-form deg for this batch's epilogue (scalar queue)
                drows = rows_pool.tile([1, 2, N], BF16, tag="drows")
                nc.scalar.dma_start(drows[:], deg_scr[b].unsqueeze(0))
                return drows

            def stage_uv_head(b, h_t, g_t):
                # bf16 g for the output matmul
                g_bf = gbf_pool.tile([128, NJ, D], BF16)
                nc.vector.tensor_copy(g_bf[:], g_t[:])
                # u/v projections (u_i = h_i . a1, v_j = g_j . a2)
                ucols = small.tile([128, NI], F32, tag="ucols")
                vcols = small.tile([128, NJ], F32, tag="vcols")
                uscr = small.tile([128, D], F32, tag="uscr")
                for ib in range(NI):
                    nc.vector.scalar_tensor_tensor(
                        uscr[:], h_t[:, ib, :], 0.0, a1bc[:],
                        OP.bypass, OP.mult, accum_out=ucols[:, ib : ib + 1],
                    )
                for jb in range(NJ):
                    nc.vector.scalar_tensor_tensor(
                        uscr[:], g_t[:, jb, :], 0.0, a2bc[:],
                        OP.bypass, OP.mult, accum_out=vcols[:, jb : jb + 1],
                    )
                # bias for the Prelu pass: v_j - BETA (per-partition)
                biasv = small.tile([128, NJ], F32, tag="biasv")
                nc.vector.tensor_scalar(
                    biasv[:], vcols[:], BETA, None, OP.subtract
                )
                # u columns to f32r, store (sync queue)
                ucols_r = small.tile([128, NI], F32R, tag="ucols_r")
                nc.vector.tensor_copy(ucols_r[:], ucols[:])
                nc.sync.dma_start(
                    urow_scr[b].rearrange("(ib p) -> p ib", p=128), ucols_r[:]
                )
                return g_bf, biasv

            def stage_uv_tail(b):
                # u row back as f32r (scalar queue; emitted after the
                # previous batch's activations so it never stalls ACT)
                urow_r = rows_pool.tile([1, N], F32R, tag="urow_r")
                nc.scalar.dma_start(urow_r[:], urow_scr[b].unsqueeze(0))
                return urow_r

            pf = prefetch(0)
            uv = stage_uv_head(0, pf[0], pf[1])
            urow_r = stage_uv_tail(0)
            for b in range(BPC):
                h_t, g_t, af = pf
                g_bf, biasv = uv

                # next batch loads first; then this batch's deg (af already
                # resident -> no SWDGE stream stall); then next batch's
                # projections
                if b + 1 < BPC:
                    pf = prefetch(b + 1)
                stage_deg_head(b, af)
                if b + 1 < BPC:
                    uv = stage_uv_head(b + 1, pf[0], pf[1])

                outT_ps = psO.tile([128, N], F32, tag="o")
                rs_ps = psR.tile([128, N], F32, tag="r")
                sT = st_pool.tile([128, NJ, N], BF16)
                pl = None
                for jb in range(NJ):
                    # scores^T for block jb: B[j', i] = BETA*a[i, j] + u_i
                    B_ps = psB.tile([128, N], F32, tag="B")
                    for ib in range(NI):
                        nc.tensor.matmul(
                            B_ps[:, ib * 128 : (ib + 1) * 128],
                            af[:, ib, jb * 128 : (jb + 1) * 128],
                            beye[:],
                            start=(ib in (0, 4)), stop=False,
                        )
                    for half in range(2):
                        fs = slice(half * 512, (half + 1) * 512)
                        nc.tensor.matmul(
                            B_ps[:, fs], ones_r[:], urow_r[:, fs],
                            start=False, stop=(half == 1),
                        )
                    if jb % 4 == 0:
                        pl = pl_pool.tile([128, 4, N], F32)
                    nc.scalar.activation(
                        pl[:, jb % 4, :], B_ps[:], AF.Prelu,
                        bias=biasv[:, jb : jb + 1], alpha=0.2,
                    )
                    if jb % 4 == 3:
                        nc.scalar.activation(
                            sT[:, jb - 3 : jb + 1, :], pl[:], AF.Exp
                        )
                        for j2 in range(jb - 3, jb + 1):
                            for half in range(2):
                                fs = slice(half * 512, (half + 1) * 512)
                                nc.tensor.matmul(
                                    outT_ps[:, fs], g_bf[:, j2, :],
                                    sT[:, j2, fs],
                                    start=(j2 == 0), stop=(j2 == NJ - 1),
                                )
                                nc.tensor.matmul(
                                    rs_ps[:, fs], ones_sq_bf[:],
                                    sT[:, j2, fs],
                                    start=(j2 == 0), stop=(j2 == NJ - 1),
                                )

                # next batch's u row load lands after this batch's ACT work
                if b + 1 < BPC:
                    urow_r_next = stage_uv_tail(b + 1)
                drows = stage_deg_tail(b)

                # ---- epilogue: out^T * (deg / rowsum), all row-form ---------
                rrow = rows1_pool.tile([128, N], F32, tag="rrow")
                nc.vector.reciprocal_approx_fast(rrow[:], rs_ps[:])
                deg_ps = psB.tile([128, N], F32, tag="B")
                for half in range(2):
                    fs = slice(half * 512, (half + 1) * 512)
                    nc.tensor.matmul(
                        deg_ps[:, fs], ones_row[:], drows[:, 0, fs],
                        start=True, stop=False,
                    )
                    nc.tensor.matmul(
                        deg_ps[:, fs], ones_row[:], drows[:, 1, fs],
                        start=False, stop=(half == 1),
                    )
                fac = rows1_pool.tile([128, N], F32, tag="fac")
                nc.vector.tensor_tensor(fac[:], deg_ps[:], rrow[:], OP.mult)
                outsbT = rows1_pool.tile([128, N], BF16, tag="outsbT")
                nc.vector.tensor_tensor(outsbT[:], outT_ps[:], fac[:], OP.mult)
                nc.sync.dma_start(out_d[b], outsbT[:])
                if b + 1 < BPC:
                    urow_r = urow_r_next

    nc.compile()
    return nc


_CACHE = {}


def _get_nc():
    if "nc" not in _CACHE:
        _CACHE["nc"] = build_bass()
    return _CACHE["nc"]


def _make_in_maps(input1, input2, adj, a1, a2):
    input1 = np.ascontiguousarray(np.asarray(input1, dtype=np.float32))
    input2 = np.ascontiguousarray(np.asarray(input2, dtype=np.float32))
    adj = np.ascontiguousarray(np.asarray(adj, dtype=np.int32))
    a1 = np.ascontiguousarray(np.asarray(a1, dtype=np.float32))
    a2 = np.ascontiguousarray(np.asarray(a2, dtype=np.float32))
    beye = np.ascontiguousarray((np.eye(128) * BETA).astype(ml_dtypes.bfloat16))
    in_maps = []
    for c in range(NCORES):
        sl = slice(c * BPC, (c + 1) * BPC)
        in_maps.append(
            {
                "input1": input1[sl],
                "input2": input2[sl],
                "adj": adj[sl],
                "a1": a1,
                "a2": a2,
                "beye": beye,
            }
        )
    return in_maps


def _gather(res):
    # device emits out^T (BPC, D, N) bf16; un-transpose + cast (layout only)
    return np.concatenate(
        [
            np.asarray(r["out"]).astype(np.float32).transpose(0, 2, 1)
            for r in res.results
        ],
        axis=0,
    )


def kernel(input1, input2, adj, a1, a2):
    nc = _get_nc()
    res = bass_utils.run_bass_kernel_spmd(
        nc, _make_in_maps(input1, input2, adj, a1, a2),
        core_ids=list(range(NCORES)),
    )
    return _gather(res)


def run_traced(input1, input2, adj, a1, a2, trace_cores=None):
    nc = _get_nc()
    res = bass_utils.run_bass_kernel_spmd(
        nc, _make_in_maps(input1, input2, adj, a1, a2),
        core_ids=list(range(NCORES)),
        trace=True,
        trace_cores=trace_cores or [0],
    )
    return _gather(res), res


# revision 11
# speedup vs baseline: 1.6090x; 1.4271x over previous
"""GAT-style masked-softmax attention kernel for Trainium2 (8 NeuronCores).

Problem (per batch b of 32):
    e   = leaky_relu(h @ a1 + (g @ a2)^T, 0.2)        # (N, M)
    att = softmax(where(adj > 0, e, -9e15), axis=-1)  # (N, M)
    out = (att * adj.sum(-1, keepdims=True)) @ g      # (N, D)

Strategy (pure data parallel over B=32 -> 4 batches/core):
  * Mask folded into the scores: softmax(where(a>0, e, -inf)) ==
    softmax(leaky_relu(u_i + v_j + BETA*a_ij - BETA)) for large BETA,
    because Prelu of a hugely negative number stays hugely negative and
    exp() of it underflows to ~0 relative to real scores.
  * adj is cast int32->bf16 during the HBM load (SWDGE cast-on-load),
    landing in natural [i, j] layout. No on-chip transpose of the mask:
    BETA*a^T is produced directly in PSUM by TensorE matmuls of af
    blocks against a BETA*I identity (transpose-by-matmul), accumulated
    with the broadcast of u (K=1 ones matmuls, exact bf16 hi+lo split).
  * ACT then does Prelu(B + (v_j - BETA)) and Exp -> s^T bf16; s^T feeds
    the output matmul (weight-stationary g) and a ones-matmul rowsum.
  * deg_i = sum_j a_ij rides a DVE tensor_scalar accumulate over af
    (natural layout, free-axis reduction); deg and u bounce through DRAM
    once per batch to become row vectors (bf16 hi/lo pairs).
  * Epilogue scales out^T by deg/rowsum row-wise; output stored bf16
    transposed, host un-transposes and casts to f32.

Self-contained: hardcodes shapes B,N,M,D = 32,1024,1024,128 on 8 cores.
"""

import sys

if "/opt/trn_rl_repo" not in sys.path:
    sys.path.insert(0, "/opt/trn_rl_repo")

import numpy as np
import ml_dtypes

import concourse.bacc as bacc
import concourse.mybir as mybir
import concourse.tile as tile
import concourse.bass_utils as bass_utils

F32 = mybir.dt.float32
F32R = mybir.dt.float32r
BF16 = mybir.dt.bfloat16
I32 = mybir.dt.int32
OP = mybir.AluOpType
AF = mybir.ActivationFunctionType

B, N, M, D = 32, 1024, 1024, 128
NCORES = 8
BPC = B // NCORES  # batches per core
NI = N // 128      # i blocks
NJ = M // 128      # j blocks
BETA = 100.0


def build_bass():
    nc = bacc.Bacc("TRN2", target_bir_lowering=False, debug=False)

    h_in = nc.dram_tensor("input1", [BPC, N, D], F32, kind="ExternalInput").ap()
    g_in = nc.dram_tensor("input2", [BPC, M, D], F32, kind="ExternalInput").ap()
    adj_in = nc.dram_tensor("adj", [BPC, N, M], I32, kind="ExternalInput").ap()
    a1_in = nc.dram_tensor("a1", [D, 1], F32, kind="ExternalInput").ap()
    a2_in = nc.dram_tensor("a2", [D, 1], F32, kind="ExternalInput").ap()
    beye_in = nc.dram_tensor("beye", [128, 128], BF16, kind="ExternalInput").ap()
    eyef_in = nc.dram_tensor("eyef", [128, 128], F32, kind="ExternalInput").ap()
    # out^T bf16: host transposes (0,2,1) + casts f32 after gather
    out_d = nc.dram_tensor("out", [BPC, D, N], BF16, kind="ExternalOutput").ap()


    with tile.TileContext(nc) as tc:
        with (
            tc.tile_pool(name="singles", bufs=1) as singles,
            tc.tile_pool(name="hg", bufs=2) as hg_pool,
            tc.tile_pool(name="gbf", bufs=2) as gbf_pool,
            tc.tile_pool(name="afp", bufs=2) as af_pool,
            tc.tile_pool(name="plp", bufs=2) as pl_pool,
            tc.tile_pool(name="st", bufs=2) as st_pool,
            tc.tile_pool(name="small", bufs=4) as small,
            tc.tile_pool(name="folds", bufs=2) as folds,
            tc.tile_pool(name="rows", bufs=2) as rows_pool,
            tc.tile_pool(name="rows1", bufs=1) as rows1_pool,
            tc.tile_pool(name="psB", bufs=2, space="PSUM") as psB,
            tc.tile_pool(name="psO", bufs=1, space="PSUM") as psO,
            tc.tile_pool(name="psR", bufs=1, space="PSUM") as psR,
        ):
            # ---- static prep ------------------------------------------------
            ones_row = singles.tile([1, 128], BF16)
            nc.vector.memset(ones_row[:], 1.0)
            ones_sq_bf = singles.tile([128, 128], BF16)
            nc.vector.memset(ones_sq_bf[:], 1.0)
            beye = singles.tile([128, 128], BF16)
            nc.sync.dma_start(beye[:], beye_in)
            eyef = singles.tile([128, 128], F32)
            nc.sync.dma_start(eyef[:], eyef_in)

            a1row = singles.tile([1, D], F32)
            nc.gpsimd.dma_start(a1row[:], a1_in.transpose((1, 0)))
            a2row = singles.tile([1, D], F32)
            nc.gpsimd.dma_start(a2row[:], a2_in.transpose((1, 0)))
            ones_f = singles.tile([1, 128], F32)
            nc.vector.memset(ones_f[:], 1.0)
            ones_r = singles.tile([1, 128], F32R)
            nc.vector.tensor_copy(ones_r[:], ones_f[:])

            a1bc = singles.tile([128, D], F32)
            a2bc = singles.tile([128, D], F32)
            bc_ps = psB.tile([128, N], F32, tag="B")
            nc.tensor.matmul(
                bc_ps[:, 0:D], ones_f[:], a1row[:], start=True, stop=False
            )
            nc.tensor.matmul(
                bc_ps[:, D : 2 * D], ones_f[:], a2row[:], start=False, stop=True
            )
            nc.vector.tensor_copy(a1bc[:], bc_ps[:, 0:D])
            nc.vector.tensor_copy(a2bc[:], bc_ps[:, D : 2 * D])


            def prefetch(b):
                h_t = hg_pool.tile([128, NI, D], F32, tag="h")
                nc.sync.dma_start(
                    h_t[:], h_in[b].rearrange("(ib p) d -> p ib d", p=128)
                )
                g_t = hg_pool.tile([128, NJ, D], F32, tag="g")
                nc.sync.dma_start(
                    g_t[:], g_in[b].rearrange("(jb p) d -> p jb d", p=128)
                )
                # adj int32 -> bf16 cast during the load (SWDGE), natural layout
                af = af_pool.tile([128, NI, M], BF16)
                nc.gpsimd.dma_start(
                    af[:], adj_in[b].rearrange("(ib p) m -> p ib m", p=128)
                )
                return h_t, g_t, af

            def stage_deg_head(b, af):
                # deg_i = sum_j a_ij for the CURRENT batch: 4 SWDGE
                # accumulate-DMAs (512B descriptors) fold j 1024 -> 256
                # (sums <= 4, exact in bf16), then one DVE reduce
                dacc = folds.tile([128, NI, 256], BF16, tag="dacc")
                nc.gpsimd.dma_start(dacc[:], af[:, :, 0:256])
                for t in range(1, 4):
                    nc.gpsimd.dma_start(
                        dacc[:], af[:, :, t * 256 : (t + 1) * 256],
                        accum_op=OP.add,
                    )
                degc = small.tile([128, NI], F32, tag="degc")
                nc.vector.tensor_reduce(
                    degc[:], dacc[:], mybir.AxisListType.X, OP.add
                )
                return degc

            def stage_deg_tail(b, degc):
                # deg columns -> row form, all on-chip: identity matmul
                # transposes [128, NI] -> [NI, 128] in PSUM, DVE rounds to
                # f32r, one 8-descriptor SBUF->SBUF DMA flattens to [1, N]
                degT = psB.tile([NI, 128], F32, tag="B")
                nc.tensor.matmul(degT[:], degc[:], eyef[:], start=True, stop=True)
                degrows8 = small.tile([NI, 128], F32R, tag="degrows8")
                nc.vector.tensor_copy(degrows8[:], degT[:])
                degrow_r = rows_pool.tile([1, N], F32R, tag="degrow_r")
                nc.scalar.dma_start(degrow_r[:], degrows8[:])
                return degrow_r

            def stage_uv_head(b, h_t, g_t):
                # bf16 g for the output matmul
                g_bf = gbf_pool.tile([128, NJ, D], BF16)
                nc.vector.tensor_copy(g_bf[:], g_t[:])
                # u/v projections (u_i = h_i . a1, v_j = g_j . a2)
                ucols = small.tile([128, NI], F32, tag="ucols")
                vcols = small.tile([128, NJ], F32, tag="vcols")
                uscr = small.tile([128, D], F32, tag="uscr")
                for ib in range(NI):
                    nc.vector.scalar_tensor_tensor(
                        uscr[:], h_t[:, ib, :], 0.0, a1bc[:],
                        OP.bypass, OP.mult, accum_out=ucols[:, ib : ib + 1],
                    )
                for jb in range(NJ):
                    nc.vector.scalar_tensor_tensor(
                        uscr[:], g_t[:, jb, :], 0.0, a2bc[:],
                        OP.bypass, OP.mult, accum_out=vcols[:, jb : jb + 1],
                    )
                # bias for the Prelu pass: v_j - BETA (per-partition)
                biasv = small.tile([128, NJ], F32, tag="biasv")
                nc.vector.tensor_scalar(
                    biasv[:], vcols[:], BETA, None, OP.subtract
                )
                return g_bf, biasv, ucols

            def stage_uv_tail(b, ucols):
                # u columns -> row form, all on-chip (same trick as deg)
                uT = psB.tile([NI, 128], F32, tag="B")
                nc.tensor.matmul(uT[:], ucols[:], eyef[:], start=True, stop=True)
                urows8 = small.tile([NI, 128], F32R, tag="urows8")
                nc.vector.tensor_copy(urows8[:], uT[:])
                urow_r = rows_pool.tile([1, N], F32R, tag="urow_r")
                nc.scalar.dma_start(urow_r[:], urows8[:])
                return urow_r

            pf = prefetch(0)
            uv = stage_uv_head(0, pf[0], pf[1])
            urow_r = stage_uv_tail(0, uv[2])
            for b in range(BPC):
                h_t, g_t, af = pf
                g_bf, biasv = uv[0], uv[1]

                # next batch loads first; then this batch's deg (af already
                # resident -> no SWDGE stream stall); then next batch's
                # projections
                if b + 1 < BPC:
                    pf = prefetch(b + 1)
                degc = stage_deg_head(b, af)
                if b + 1 < BPC:
                    uv = stage_uv_head(b + 1, pf[0], pf[1])

                outT_ps = psO.tile([128, N], F32, tag="o")
                rs_ps = psR.tile([128, N], F32, tag="r")
                sT = st_pool.tile([128, NJ, N], BF16)
                pl = None
                for jb in range(NJ):
                    # scores^T for block jb: B[j', i] = BETA*a[i, j] + u_i
                    B_ps = psB.tile([128, N], F32, tag="B")
                    for ib in range(NI):
                        nc.tensor.matmul(
                            B_ps[:, ib * 128 : (ib + 1) * 128],
                            af[:, ib, jb * 128 : (jb + 1) * 128],
                            beye[:],
                            start=(ib in (0, 4)), stop=False,
                        )
                    for half in range(2):
                        fs = slice(half * 512, (half + 1) * 512)
                        nc.tensor.matmul(
                            B_ps[:, fs], ones_r[:], urow_r[:, fs],
                            start=False, stop=(half == 1),
                        )
                    if jb % 4 == 0:
                        pl = pl_pool.tile([128, 4, N], F32)
                    nc.scalar.activation(
                        pl[:, jb % 4, :], B_ps[:], AF.Prelu,
                        bias=biasv[:, jb : jb + 1], alpha=0.2,
                    )
                    if jb % 4 == 3:
                        nc.scalar.activation(
                            sT[:, jb - 3 : jb + 1, :], pl[:], AF.Exp
                        )
                        for j2 in range(jb - 3, jb + 1):
                            for half in range(2):
                                fs = slice(half * 512, (half + 1) * 512)
                                nc.tensor.matmul(
                                    outT_ps[:, fs], g_bf[:, j2, :],
                                    sT[:, j2, fs],
                                    start=(j2 == 0), stop=(j2 == NJ - 1),
                                )
                                nc.tensor.matmul(
                                    rs_ps[:, fs], ones_sq_bf[:],
                                    sT[:, j2, fs],
                                    start=(j2 == 0), stop=(j2 == NJ - 1),
                                )

                # next batch's u row transpose lands after this batch's MMs
                if b + 1 < BPC:
                    urow_r_next = stage_uv_tail(b + 1, uv[2])
                degrow_r = stage_deg_tail(b, degc)

                # ---- epilogue: out^T * (deg / rowsum), all row-form ---------
                rrow = rows1_pool.tile([128, N], F32, tag="rrow")
                nc.vector.reciprocal_approx_fast(rrow[:], rs_ps[:])
                deg_ps = psB.tile([128, N], F32, tag="B")
                for half in range(2):
                    fs = slice(half * 512, (half + 1) * 512)
                    nc.tensor.matmul(
                        deg_ps[:, fs], ones_r[:], degrow_r[:, fs],
                        start=True, stop=(half == 1),
                    )
                fac = rows1_pool.tile([128, N], F32, tag="fac")
                nc.vector.tensor_tensor(fac[:], deg_ps[:], rrow[:], OP.mult)
                outsbT = rows1_pool.tile([128, N], BF16, tag="outsbT")
                nc.vector.tensor_tensor(outsbT[:], outT_ps[:], fac[:], OP.mult)
                nc.sync.dma_start(out_d[b], outsbT[:])
                if b + 1 < BPC:
                    urow_r = urow_r_next

    nc.compile()
    return nc


_CACHE = {}


def _get_nc():
    if "nc" not in _CACHE:
        _CACHE["nc"] = build_bass()
    return _CACHE["nc"]


def _make_in_maps(input1, input2, adj, a1, a2):
    input1 = np.ascontiguousarray(np.asarray(input1, dtype=np.float32))
    input2 = np.ascontiguousarray(np.asarray(input2, dtype=np.float32))
    adj = np.ascontiguousarray(np.asarray(adj, dtype=np.int32))
    a1 = np.ascontiguousarray(np.asarray(a1, dtype=np.float32))
    a2 = np.ascontiguousarray(np.asarray(a2, dtype=np.float32))
    beye = np.ascontiguousarray((np.eye(128) * BETA).astype(ml_dtypes.bfloat16))
    eyef = np.ascontiguousarray(np.eye(128, dtype=np.float32))
    in_maps = []
    for c in range(NCORES):
        sl = slice(c * BPC, (c + 1) * BPC)
        in_maps.append(
            {
                "input1": input1[sl],
                "input2": input2[sl],
                "adj": adj[sl],
                "a1": a1,
                "a2": a2,
                "beye": beye,
                "eyef": eyef,
            }
        )
    return in_maps


def _gather(res):
    # device emits out^T (BPC, D, N) bf16; un-transpose + cast (layout only)
    return np.concatenate(
        [
            np.asarray(r["out"]).astype(np.float32).transpose(0, 2, 1)
            for r in res.results
        ],
        axis=0,
    )


def kernel(input1, input2, adj, a1, a2):
    nc = _get_nc()
    res = bass_utils.run_bass_kernel_spmd(
        nc, _make_in_maps(input1, input2, adj, a1, a2),
        core_ids=list(range(NCORES)),
    )
    return _gather(res)


def run_traced(input1, input2, adj, a1, a2, trace_cores=None):
    nc = _get_nc()
    res = bass_utils.run_bass_kernel_spmd(
        nc, _make_in_maps(input1, input2, adj, a1, a2),
        core_ids=list(range(NCORES)),
        trace=True,
        trace_cores=trace_cores or [0],
    )
    return _gather(res), res
